# revision 16
# baseline (speedup 1.0000x reference)
"""Trainium2 Bass kernel for nn_Attention (b=4, n=2048, d=1024, 16 heads x 64).

Strategy (8 NeuronCores, zero collectives):
  core i -> batch b = i//2, query-row half h = i%2.
  Each core computes K/V for ALL 2048 positions of its batch (kv projection is
  duplicated across the core pair; ~25% extra PE work buys zero communication),
  and attention + output projection for its 1024 query rows.

  Fused head-pair pipeline (v3): the softmax exp chain on the ACT engine
  (33.5M elems @ 153.6 G/s ~= 220us/core) and the PE matmul stream (~1.6k
  512-col slots ~= 350us/core) are the two near-critical engines. The per-
  engine FIFOs execute in emission order, so overlap must be *emitted*:
  projection matmuls for head-pair hp+1 are chopped into ~3-matmul chunks by
  a generator and interleaved between every score/PV group of head-pair hp's
  attention. The PE then always has independent work queued while it waits
  for the exp of the next k-block, and the ACT engine runs exp back-to-back.

  Device pipeline per core (all matmuls bf16, fp32 PSUM accumulation):
    - kT/qT in transposed [chan, pos] layout via (W^T X^T); RoPE applied as
      y = cos*x + sin*(PermSign @ x) with the PermSign matmul on the PE,
      elementwise on GpSimd/DVE, PSUM evictions on DVE (ACT stays exp-only).
    - v in natural [pos, chan] layout with a ones-column per head (so the PV
      matmul also produces the softmax row-sums for free).
    - Scores S^T[k,q] = kT_h^T @ qT_h per head; two heads run concurrently
      via 64-row array tiling. exp on ACT with the 1/sqrt(dh) scale folded
      in, batched over 2 PSUM banks per instruction. P@V with v65 stationary
      accumulates O^T pieces [65, 512] over k-blocks (row 64 = denominator);
      PV for k-block kb is emitted 2 exp-instructions behind its scores so
      the PE FIFO never waits on the ACT engine.
    - Output projection straight from O^T, bias added during fp32 eviction;
      the qg=0 half is interleaved into the last head-pair's attention.
"""

import numpy as np
import ml_dtypes

BF16 = ml_dtypes.bfloat16

B, N, D = 4, 2048, 1024
HEADS, DH, ROT = 16, 64, 32
INNER = HEADS * DH          # 1024
NH = N // 2                 # query rows per core
KC = D // 128               # 8 contraction chunks
MC = INNER // 128           # 8 channel chunks (head pairs)
NB = N // 128               # 16 position blocks
SCALE = DH ** -0.5
N_CORES = 8

_CACHE = {}


def _build_nc():
    import concourse.bacc as bacc
    import concourse.mybir as mybir
    import concourse.tile as tile

    dt = mybir.dt
    f32, bf16 = dt.float32, dt.bfloat16
    Alu = mybir.AluOpType
    Act = mybir.ActivationFunctionType

    nc = bacc.Bacc("TRN2", target_bir_lowering=False, debug=False)

    # DRAM parameters (per-core shards; layouts documented in prepare_in_maps)
    xt_d = nc.dram_tensor("xt", [128, KC, N], bf16, kind="ExternalInput")
    wk_d = nc.dram_tensor("wk", [128, MC, KC, 128], bf16, kind="ExternalInput")
    wq_d = nc.dram_tensor("wq", [128, MC, KC, 128], bf16, kind="ExternalInput")
    wv_d = nc.dram_tensor("wv", [128, 2, KC, 512], bf16, kind="ExternalInput")
    wo_d = nc.dram_tensor("wo", [128, MC, D], bf16, kind="ExternalInput")
    bb_d = nc.dram_tensor("bb", [128, D], bf16, kind="ExternalInput")
    cos_d = nc.dram_tensor("cosk", [128, N], bf16, kind="ExternalInput")
    sin_d = nc.dram_tensor("sink", [128, N], bf16, kind="ExternalInput")
    psgn_d = nc.dram_tensor("psgn", [128, 128], bf16, kind="ExternalInput")
    iden_d = nc.dram_tensor("iden", [128, 128], bf16, kind="ExternalInput")
    out_d = nc.dram_tensor("out", [NH, D], f32, kind="ExternalOutput")

    with tile.TileContext(nc) as tc:
        with (
            # ---- resident for the whole kernel ----
            tc.tile_pool(name="const", bufs=1) as constp,
            tc.tile_pool(name="ktr", bufs=1) as ktrp,
            tc.tile_pool(name="qtr", bufs=1) as qtrp,
            tc.tile_pool(name="v65", bufs=1) as v65p,
            tc.tile_pool(name="ot", bufs=1) as otp,
            tc.tile_pool(name="xt", bufs=1) as xtp,
            tc.tile_pool(name="wo", bufs=1) as wop,
            tc.tile_pool(name="pt", bufs=6) as ptp,
            tc.tile_pool(name="rv", bufs=1) as rvp,
            tc.tile_pool(name="wslice", bufs=2) as wsp,
            tc.tile_pool(name="wv", bufs=2) as wvp,
            tc.tile_pool(name="tmp", bufs=2) as tmpp,
            tc.tile_pool(name="outf", bufs=2) as outfp,
            # ---- PSUM ----
            tc.tile_pool(name="ps512", bufs=2, space="PSUM") as psp,
            tc.tile_pool(name="pss", bufs=2, space="PSUM") as pssp,
            tc.tile_pool(name="pso", bufs=2, space="PSUM") as psop,
        ):
            cos_sb = constp.tile([128, N], bf16, tag="cos")
            sin_sb = constp.tile([128, N], bf16, tag="sin")
            psgn_sb = constp.tile([128, 128], bf16, tag="psgn")
            ones_pad = constp.tile([128, 128], bf16, tag="ones_pad")
            bb_sb = constp.tile([128, D], bf16, tag="bb")
            nc.sync.dma_start(psgn_sb[:], psgn_d.ap())
            nc.vector.memset(ones_pad[:], 0.0)
            nc.vector.memset(ones_pad[0:1, :], 1.0)
            # reciprocal row for softmax denominators: only partition 0 is
            # ever written; the rest are zeroed once so the broadcast matmul
            # (ones_pad has zeros there) sees no NaN garbage.
            rv = constp.tile([128, 512], bf16, tag="rv")
            nc.vector.memset(rv[:], 0.0)

            kTr = ktrp.tile([128, MC, N], bf16, tag="kTr")
            qTr = qtrp.tile([128, MC, NH], bf16, tag="qTr")
            v65 = v65p.tile([128, NB, HEADS * 65], bf16, tag="v65")
            oT = otp.tile([128, MC, NH], bf16, tag="oT")
            xt = xtp.tile([128, KC, N], bf16, tag="xt")
            wo_sb = wop.tile([128, MC, D], bf16, tag="wo")

            # ones column per head inside v65 (softmax denominator trick)
            v65_g = v65[:].rearrange("p b (g s) -> p b g s", s=65)
            nc.vector.memset(v65_g[:, :, :, 64:65], 1.0)

            # input DMAs in dependency order: everything attn(0,*) needs
            # first; wo/bb (outproj-only) trail the prologue weights
            for kc in range(KC):
                nc.sync.dma_start(xt[:, kc], xt_d.ap()[:, kc])
            nc.sync.dma_start(cos_sb[:], cos_d.ap())
            nc.sync.dma_start(sin_sb[:], sin_d.ap())

            def load_tail_weights():
                nc.sync.dma_start(bb_sb[:], bb_d.ap())
                for m in range(MC):
                    nc.sync.dma_start(wo_sb[:, m], wo_d.ap()[:, m])

            def rope_b(dst, raw, cos_ap, sin_ap):
                """Phase B of RoPE: dst = cos*raw + sin*(PermSign @ raw).

                Emitted one j-group after raw was evicted, so the PE FIFO
                reaches the PermSign matmul long after the DVE copy landed."""
                ps_z = psp.tile([128, 512], f32, tag="ps512", name="ps_z")
                nc.tensor.matmul(
                    ps_z[:], psgn_sb[:], raw[:], start=True, stop=True,
                )
                zs = tmpp.tile([128, 512], bf16, tag="zs", name="zs")
                nc.vector.scalar_tensor_tensor(
                    out=zs[:], in0=ps_z[:], scalar=0.0, in1=sin_ap,
                    op0=Alu.bypass, op1=Alu.mult,
                )
                nc.gpsimd.tensor_mul(out=dst, in0=raw[:], in1=cos_ap)
                nc.gpsimd.tensor_add(out=dst, in0=dst, in1=zs[:])

            # ------- projection / outproj work units (generators) -------
            def gen_proj(dst, m, w_m, jlo, jhi):
                """kT/qT projection for m-chunk, j-tiles [jlo, jhi)."""
                prev = None
                for j in range(jlo, jhi):
                    ps = psp.tile([128, 512], f32, tag="ps512", name="ps_pj")
                    for kc in range(KC):
                        nc.tensor.matmul(
                            ps[:],
                            w_m[:, kc],
                            xt[:, kc, j * 512:(j + 1) * 512],
                            start=(kc == 0),
                            stop=(kc == KC - 1),
                        )
                        if kc % 3 == 2:
                            yield
                    sl = slice(j * 512, (j + 1) * 512)
                    raw = tmpp.tile([128, 512], bf16, tag="raw", name="raw")
                    nc.vector.tensor_copy(raw[:], ps[:])
                    if prev is not None:
                        rope_b(dst[:, m, prev[1]], prev[0],
                               cos_sb[:, prev[1]], sin_sb[:, prev[1]])
                    yield
                    prev = (raw, sl)
                rope_b(dst[:, m, prev[1]], prev[0],
                       cos_sb[:, prev[1]], sin_sb[:, prev[1]])
                yield

            def gen_v(vc, wv_vc, nbs):
                for nb in nbs:
                    ps = psp.tile([128, 512], f32, tag="ps512", name="ps_v")
                    for kc in range(KC):
                        nc.tensor.matmul(
                            ps[:],
                            xt[:, kc, nb * 128:(nb + 1) * 128],
                            wv_vc[:, kc],
                            start=(kc == 0),
                            stop=(kc == KC - 1),
                        )
                        if kc % 3 == 2:
                            yield
                    dst = v65_g[:, nb, vc * 8:(vc + 1) * 8, 0:64]
                    src = ps[:].rearrange("p (g s) -> p g s", s=64)
                    nc.vector.tensor_copy(dst, src)
                    yield

            def gen_outproj(nbs):
                for nb in nbs:
                    for dc in range(2):
                        ps = psp.tile([128, 512], f32, tag="ps512", name="ps_op")
                        for ic in range(MC):
                            nc.tensor.matmul(
                                ps[:],
                                oT[:, ic, nb * 128:(nb + 1) * 128],
                                wo_sb[:, ic, dc * 512:(dc + 1) * 512],
                                start=(ic == 0),
                                stop=(ic == MC - 1),
                            )
                            if ic % 3 == 2:
                                yield
                        outf = outfp.tile([128, 512], f32, tag="outf",
                                          name="outf")
                        nc.vector.tensor_tensor(
                            out=outf[:], in0=ps[:],
                            in1=bb_sb[:, dc * 512:(dc + 1) * 512],
                            op=Alu.add,
                        )
                        nc.sync.dma_start(
                            out_d.ap()[nb * 128:(nb + 1) * 128,
                                       dc * 512:(dc + 1) * 512],
                            outf[:],
                        )
                        yield

            def chain(*gens):
                for g in gens:
                    yield from g

            class Work:
                """Pull-driven emitter over a chunked work generator."""
                def __init__(self, gen):
                    self.gen = gen
                    self.left = True

                def pull(self, n):
                    for _ in range(n):
                        if not self.left:
                            return
                        try:
                            next(self.gen)
                        except StopIteration:
                            self.left = False
                            return

                def drain(self):
                    while self.left:
                        self.pull(1)

            # ------------------- attention -------------------
            def attn(hp, qg, work, per_kb):
                qsl = slice(qg * 512, (qg + 1) * 512)
                # O^T pieces [65, 512]: rows 0:64 = head channels,
                # row 64 = softmax denominator (ones column of v65)
                ps_o = [
                    psop.tile([65, 512], f32, tag="pso", name="ps_o")
                    for _ in range(2)
                ]
                pts = []

                def emit_pv(kb):
                    for h in range(2):
                        hg = 2 * hp + h
                        nc.tensor.matmul(
                            ps_o[h][:],
                            v65_g[:, kb, hg],
                            pts[kb][:, h * 512:(h + 1) * 512],
                            start=(kb == 0),
                            stop=(kb == NB - 1),
                        )

                # 2 k-blocks per iteration: scores batched back-to-back so
                # the PE pays the row-tiled<->full-array LDWEIGHTS
                # serialization once per pair of k-blocks, not once per kb
                for kb2 in range(0, NB, 2):
                    for kb in (kb2, kb2 + 1):
                        ksl = slice(kb * 128, (kb + 1) * 128)
                        ps_s = pssp.tile([128, 1024], f32, tag="pss",
                                         name="ps_s")
                        for h in range(2):
                            pr = slice(h * 64, (h + 1) * 64)
                            nc.tensor.matmul(
                                ps_s[:, h * 512:(h + 1) * 512],
                                kTr[pr, hp, ksl],
                                qTr[pr, hp, qsl],
                                start=True, stop=True,
                            )
                        pt = ptp.tile([128, 1024], bf16, tag="pt", name="pt")
                        nc.scalar.activation(pt[:], ps_s[:], Act.Exp,
                                             scale=SCALE)
                        pts.append(pt)
                    work.pull(per_kb)
                    # PV trails the exp chain by 3-4 k-blocks so the PE FIFO
                    # reaches each PV after its exp has already finished
                    if kb2 >= 4:
                        emit_pv(kb2 - 4)
                        emit_pv(kb2 - 3)
                    work.pull(per_kb)
                for kb in range(NB - 4, NB):
                    emit_pv(kb)
                    work.pull(1)
                # normalize: oT[ch, q] = piece[ch, q] * (1/den[q]); den row
                # broadcast across partitions via the ones_pad matmul
                for h in range(2):
                    hg = 2 * hp + h
                    ic, ph = hg // 2, (hg % 2) * 64
                    den_sb = rvp.tile([1, 512], f32, tag="den_sb",
                                      name="den_sb")
                    nc.vector.tensor_copy(den_sb[:], ps_o[h][64:65, :])
                    rvf = rvp.tile([1, 512], f32, tag="rvf", name="rvf")
                    nc.vector.reciprocal_approx_fast(rvf[:], den_sb[:])
                    nc.vector.tensor_copy(rv[0:1, :], rvf[:])
                    bc = psp.tile([128, 512], f32, tag="ps512", name="ps_bc")
                    nc.tensor.matmul(
                        bc[:], ones_pad[:], rv[:], start=True, stop=True,
                    )
                    bcs = tmpp.tile([64, 512], bf16, tag="bcs", name="bcs")
                    nc.vector.tensor_copy(bcs[:], bc[0:64, :])
                    nc.vector.scalar_tensor_tensor(
                        out=oT[ph:ph + 64, ic, qsl],
                        in0=ps_o[h][0:64, :], scalar=0.0, in1=bcs[:],
                        op0=Alu.bypass, op1=Alu.mult,
                    )
                    work.pull(1)

            # ---------------- weight loads ----------------
            def load_wk(m):
                w = wsp.tile([128, KC, 128], bf16, tag="wk_m", name="wk_m")
                nc.sync.dma_start(w[:], wk_d.ap()[:, m])
                return w

            def load_wq(m):
                w = wsp.tile([128, KC, 128], bf16, tag="wq_m", name="wq_m")
                nc.sync.dma_start(w[:], wq_d.ap()[:, m])
                return w

            def load_wv(vc):
                w = wvp.tile([128, KC, 512], bf16, tag="wv_vc", name="wv_vc")
                nc.sync.dma_start(w[:], wv_d.ap()[:, vc])
                return w

            # ---------------- fused pipeline ----------------
            # prologue: only what attn(0,0)'s first scores need (PE fill)
            wk_c = load_wk(0)
            wq_c = load_wq(0)
            wv0 = load_wv(0)
            Work(chain(
                gen_proj(kTr, 0, wk_c, 0, N // 512),
                gen_proj(qTr, 0, wq_c, 0, NH // 512),
            )).drain()
            load_tail_weights()

            # steady stages: attention(hp) interleaved with projections for
            # hp+1 (v(0) hides under attn(0,0); v(1) spreads over stages
            # 0-3; the qg=0 output projection hides under attn(7, qg=1))
            v1_nbs = {0: range(0, 4), 1: range(4, 8), 2: range(8, 12),
                      3: range(12, 16)}
            wv1 = None
            for hp in range(MC):
                gens = []
                if hp + 1 < MC:
                    wk_n = load_wk(hp + 1)
                    wq_n = load_wq(hp + 1)
                    gens.append(gen_proj(kTr, hp + 1, wk_n, 0, N // 512))
                    gens.append(gen_proj(qTr, hp + 1, wq_n, 0, NH // 512))
                if hp == 0:
                    wv1 = load_wv(1)
                if hp in v1_nbs:
                    gens.append(gen_v(1, wv1, v1_nbs[hp]))
                if hp == 0:
                    work = Work(gen_v(0, wv0, range(NB)))
                    attn(hp, 0, work, 3)
                    work.drain()
                    work = Work(chain(*gens))
                    attn(hp, 1, work, 2)
                    work.drain()
                elif hp == MC - 1:
                    work = Work(chain(*gens))
                    attn(hp, 0, work, 1)
                    work.drain()
                    work = Work(gen_outproj(range(4)))
                    attn(hp, 1, work, 2)
                    work.drain()
                else:
                    work = Work(chain(*gens))
                    attn(hp, 0, work, 1)
                    attn(hp, 1, work, 1)
                    work.drain()

            Work(gen_outproj(range(4, 8))).drain()
    nc.compile()
    return nc


def get_nc():
    if "nc" not in _CACHE:
        _CACHE["nc"] = _build_nc()
    return _CACHE["nc"]


def prepare_in_maps(queries, Wq, Wkv, Wout, bout):
    """Host-side staging: shard + pre-layout + pre-cast (bf16)."""
    queries = np.asarray(queries, dtype=np.float32)
    Wq = np.asarray(Wq, dtype=np.float32)
    Wkv = np.asarray(Wkv, dtype=np.float32)
    Wout = np.asarray(Wout, dtype=np.float32)
    bout = np.asarray(bout, dtype=np.float32)

    def chunkT(W, cols):  # [D, cols] -> [128, cols//128, KC, 128]
        return np.ascontiguousarray(
            W.reshape(KC, 128, cols // 128, 128).transpose(1, 2, 0, 3)
        ).astype(BF16)

    wk = chunkT(Wkv[:, :INNER], INNER)
    wq = chunkT(Wq, INNER)
    wv = np.ascontiguousarray(
        Wkv[:, INNER:].reshape(KC, 128, 2, 512).transpose(1, 2, 0, 3)
    ).astype(BF16)
    wo = np.ascontiguousarray(
        Wout.reshape(MC, 128, D).transpose(1, 0, 2)
    ).astype(BF16)
    bb = np.ascontiguousarray(np.broadcast_to(bout, (128, D))).astype(BF16)

    psgn = np.zeros((128, 128), np.float32)
    for base in (0, 64):
        for i in range(ROT // 2):
            psgn[base + 2 * i + 1, base + 2 * i] = -1.0
            psgn[base + 2 * i, base + 2 * i + 1] = 1.0
    psgn = psgn.astype(BF16)
    iden = np.eye(128, dtype=np.float32).astype(BF16)

    inv_freq = (10000.0 ** (-np.arange(0, ROT, 2, dtype=np.float32) / ROT))

    in_maps = []
    for core in range(N_CORES):
        b, h = core // 2, core % 2
        order = np.concatenate([
            np.arange(h * NH, (h + 1) * NH),
            np.arange((1 - h) * NH, (2 - h) * NH),
        ])
        xp = queries[b][order]                      # [N, D]
        xt = np.ascontiguousarray(
            xp.T.reshape(KC, 128, N).transpose(1, 0, 2)
        ).astype(BF16)
        pos = order.astype(np.float32)
        ang = pos[None, :] * inv_freq[:, None]      # [16, N]
        c16, s16 = np.cos(ang), np.sin(ang)
        cosk = np.ones((128, N), np.float32)
        sink = np.zeros((128, N), np.float32)
        for base in (0, 64):
            for c in range(ROT):
                cosk[base + c] = c16[c // 2]
                sink[base + c] = s16[c // 2]
        in_maps.append({
            "xt": xt, "wk": wk, "wq": wq, "wv": wv, "wo": wo, "bb": bb,
            "cosk": cosk.astype(BF16), "sink": sink.astype(BF16),
            "psgn": psgn, "iden": iden,
        })
    return in_maps


def gather(results):
    out = np.empty((B, N, D), np.float32)
    for core in range(N_CORES):
        b, h = core // 2, core % 2
        out[b, h * NH:(h + 1) * NH] = results[core]["out"]
    return out


def kernel(queries, Wq, Wkv, Wout, bout):
    from concourse.bass_utils import run_bass_kernel_spmd

    nc = get_nc()
    in_maps = prepare_in_maps(queries, Wq, Wkv, Wout, bout)
    res = run_bass_kernel_spmd(nc, in_maps, core_ids=list(range(N_CORES)))
    return gather(res.results)
